# revision 54
# baseline (speedup 1.0000x reference)
"""Multi-head attention Bass/Tile kernel for 8 TRN2 NeuronCores.

Problem: nn_MultiHeadAttention (B=4, T1=T2=2048, d_model=256, d_key=32, H=8,
per-head value dim = d_model).  Reference math (no score scaling, no mask):

    k = key   @ WK^T + bk           [B, T1, 256]   (head h -> cols 32h..32h+32)
    q = query @ WQ^T + bq           [B, T2, 256]
    v = value @ WV^T + bv           [B, T1, 2048]  (head h -> cols 256h..256h+256)
    scores_h = k_h q_h^T            [T1, T2]
    attn = softmax over T1 (keys)
    emb_h = attn^T v_h              [T2, 256]
    out = emb' @ WO^T + bo          emb' channel c = d*8 + h (d outer, h inner)

Key algebraic restructure vs the direct form: since softmax weights sum to 1,

    out[q, :] = sum_h (attn_h^T @ val) @ G_h^T + bo
    G_h = WO_h @ WV_h   (host-folded weight product, [256, 256] per head)
    bo  = wob + sum_h WO_h @ bv_h   (host-folded bias)

so the per-head value projection (val @ WV_h^T) and the WO matmul collapse
into a single small per-head GEMM against the unnormalized attention-weighted
value sum P_h[c, q] = sum_s val[s, c] E_h[s, q], normalized at the end by the
per-query 1/denominator.  This removes the v-projection (64 MB intermediate,
~27 us of PE time per core) and all wo transposes from the device program.

Sharding: core c handles (batch b = c//2, query half qs = c%2) -> each core
computes the full output slice out[b, qs*1024:(qs+1)*1024, :].  No collectives.

Layout strategy: the host ships bf16 inputs pre-transposed so the device does
ZERO transposes and ZERO dtype casts:
  - keyT/qryT [256, s] channel-major (for the k/q projections)
  - val [s, 256] natural (stationary operand of the P matmul contracts over s)
  - gt [h*256+c, e] = G_h^T rows (rhs of the output GEMM)

Per-core pipeline (units u = (head, 512-query-chunk), 16 units, half-unit
software lag):
  - scores_h = kT_h^T qT_h on PE into PSUM [s, q] tiles (bf16, tile_position
    packs the 32-row stationary into PE quadrant rows 32*(h%4)).
  - E = exp(scores) on ACT (no max subtraction; fp32 range is plenty), the
    only ACT work in the kernel so the Exp table is loaded exactly once.
  - P(u): 32 chained matmuls val-stationary x E -> psum [c, 512q]; the early
    half runs inside unit u itself, the late half + denominators (ones-
    matmul, out free size 1) in unit u+1, so the pipeline drains half a unit
    after the last score.
  - copies P->SBUF bf16 (ct0 on DVE, ct1 on ACT) + reciprocal on DVE in
    unit u+1, then out2(h): psum [q, e] = P_h^T G_h^T, scaled by recip and
    accumulated into acc on DVE (scalar_tensor_tensor), bias at h==0.
  - a chain of dummy matmuls on a memset tile pre-ramps the PE p-state
    during the initial DMAs, so real work starts at the full 2.4 GHz clock
    instead of spending its first 3 us at the mid-p-state clock.
  - k/q projections for the second head group are deferred into unit 0's
    task list, marked logically-later via tile_wait_until so the Tile list
    scheduler cannot hoist them ahead of the unit-0 scores in the in-order
    PE queue (their staggered key chunks would stall the pipe); DMA chunks
    are interleaved across the sync/scalar HWDGE queues in compute need-
    order, with gt/biases on the gpsimd SWDGE queue.
  - tail: the last unit finishes its ct0 chain first (copy overlaps ct1),
    ct1 copy rides ACT, and the final output DMAs are split/queued so the
    last transfer is a single 128-query tile.

Measured (TimelineSim cost model, the graded metric): 196038 ns vs 304140 ns
for the previous kernel (1.55x), rel err 8.0e-3 (budget 2e-2).  PE busy is
~184 us = the algorithmic floor for this formulation (scores 131k cycles +
P 262k + out2 33k + proj 12k at full MAC efficiency); residual time is the
irreducible first-DMA latency chain (~5 us) and the final
scale->output-DMA->drain chain (~4.5 us).

kernel(**inputs) takes the FULL unsharded inputs and returns the full output.
"""

import numpy as np
import ml_dtypes
from contextlib import ExitStack

import concourse.bass as bass
import concourse.bacc as bacc
import concourse.mybir as mybir
import concourse.tile as tile
from concourse.bass_utils import run_bass_kernel_spmd

P = 128
B, T1, T2, DM, DK, H = 4, 2048, 2048, 256, 32, 8
QSH = T2 // 2  # queries per core
N_CORES = 8

F32 = mybir.dt.float32
BF16 = mybir.dt.bfloat16
AF = mybir.ActivationFunctionType
BF = ml_dtypes.bfloat16

ST = T1 // P        # 16 key/seq tiles
QT = QSH // P       # 8 query tiles per core
QC = 512            # query chunk (PSUM free dim)
NQC = QSH // QC     # 2 query chunks
NU = H * NQC        # 16 pipeline units


def _build_bass():
    nc = bacc.Bacc("TRN2", target_bir_lowering=False, debug=False)

    keyT = nc.dram_tensor("keyT", [DM, T1], BF16, kind="ExternalInput").ap()
    qryT = nc.dram_tensor("qryT", [DM, QSH], BF16, kind="ExternalInput").ap()
    val = nc.dram_tensor("val_x", [T1, DM], BF16, kind="ExternalInput").ap()
    wkq = nc.dram_tensor("wkq", [2 * DM, DM], BF16, kind="ExternalInput").ap()
    wkb = nc.dram_tensor("wkb", [DM], F32, kind="ExternalInput").ap()
    wqb = nc.dram_tensor("wqb", [DM], F32, kind="ExternalInput").ap()
    gt = nc.dram_tensor("gt", [H * DM, DM], BF16, kind="ExternalInput").ap()
    bo = nc.dram_tensor("bo", [DM], F32, kind="ExternalInput").ap()
    out = nc.dram_tensor("out_y", [QSH, DM], F32, kind="ExternalOutput").ap()

    with tile.TileContext(nc, pool_alloc_mode="queue") as tc:
        with ExitStack() as ctx:
            _body(ctx, tc, keyT, qryT, val, wkq, wkb, wqb, gt, bo, out)
    nc.compile()
    return nc


def _body(ctx, tc, keyT, qryT, val, wkq, wkb, wqb, gt, bo, out):
    nc = tc.nc
    mult, add = mybir.AluOpType.mult, mybir.AluOpType.add
    consts = ctx.enter_context(tc.tile_pool(name="consts", bufs=1))
    main = ctx.enter_context(tc.tile_pool(name="main", bufs=1))
    # One PSUM pool, 8 banks total:
    #   SC: [128,2,512] f32 x2  (4 banks) score tiles
    #   PP: [128,512]  f32 x2   (2 banks) k-proj, then P accumulators
    #   OU: [128,512]  f32 x2   (2 banks) q-proj, denominators, out2 psums
    pP = ctx.enter_context(tc.tile_pool(name="pP", bufs=1, space="PSUM"))

    # touch the ACT Exp table early; exp is the ONLY ACT op in the kernel so
    # the table is loaded exactly once, off the critical path
    actwarm = consts.tile([1, 1], F32)
    nc.scalar.activation(out=actwarm, in_=actwarm, func=AF.Exp)
    warm_sb = consts.tile([P, 64], BF16)
    nc.vector.memset(warm_sb, 0.0)
    ones_bf = consts.tile([P, 1], BF16)
    nc.vector.memset(ones_bf, 1.0)
    wk_b = consts.tile([P, 2], F32)
    wq_b = consts.tile([P, 2], F32)
    bo_row = consts.tile([1, DM], F32)
    bo_bc = consts.tile([P, DM], F32)

    # persistent SBUF tensors
    keyT_sb = main.tile([P, 2, T1], BF16)        # [c%128, c//128, s]
    qryT_sb = main.tile([P, 2, QSH], BF16)
    wkq_sb = main.tile([P, 4, DM], BF16)         # [c%128, c//128 (+2 for wq), ch]
    wkT_sb = wkq_sb[:, 0:2, :]
    wqT_sb = wkq_sb[:, 2:4, :]
    val_sb = main.tile([P, ST, DM], BF16)        # [s%128, s//128, c]
    gt_sb = main.tile([P, 2 * H, DM], BF16)      # [c%128, 2h+(c//128), e]
    kT = main.tile([P, 2, T1], BF16)             # [32*(h%4)+d, h//4, s]
    qT = main.tile([P, 2, QSH], BF16)
    E_ring = [main.tile([P, ST, QC], BF16, name=f"Er{i}") for i in range(2)]
    P_sb = [main.tile([P, 2, QSH], BF16, name=f"Psb{i}") for i in range(2)]
    recip_t = main.tile([P, H, QT], F32)         # [q%128, h, q//128] = 1/denom
    acc = main.tile([P, QT, DM], F32)            # output accumulator [q, e]

    # ---------------- DMA loads, priority-ordered ---------------------------
    keyT_src = keyT.rearrange("(t p) s -> p t s", p=P)
    qryT_src = qryT.rearrange("(t p) s -> p t s", p=P)
    val_src = val.rearrange("(n p) c -> p n c", p=P)
    gt_src = gt.rearrange("(n p) e -> p n e", p=P)
    # the HWDGE queues drain in near-alternation on the shared HWDGE /
    # DMA_ENGINES devices; assign chunks so that alternation matches the
    # order compute needs them: wkq, key0, qry0, qry1, key1, key3, key2, ...
    # (key0 gates the very first matmul, so it rides the scalar queue whose
    # first slot clears right after wkq).  gt goes via the gpsimd SWDGE
    # queue, off the HWDGE device entirely.
    nc.sync.dma_start(out=wkq_sb, in_=wkq.rearrange("(t p) c -> p t c", p=P))
    nc.scalar.dma_start(out=keyT_sb[:, :, 0:QC], in_=keyT_src[:, :, 0:QC])
    nc.sync.dma_start(out=qryT_sb[:, :, 0:QC], in_=qryT_src[:, :, 0:QC])
    nc.scalar.dma_start(out=qryT_sb[:, :, QC:2 * QC],
                        in_=qryT_src[:, :, QC:2 * QC])
    nc.sync.dma_start(out=keyT_sb[:, :, QC:2 * QC],
                      in_=keyT_src[:, :, QC:2 * QC])
    nc.scalar.dma_start(out=keyT_sb[:, :, 3 * QC:4 * QC],
                        in_=keyT_src[:, :, 3 * QC:4 * QC])
    nc.sync.dma_start(out=keyT_sb[:, :, 2 * QC:3 * QC],
                      in_=keyT_src[:, :, 2 * QC:3 * QC])
    for i in range(2):
        nc.sync.dma_start(out=val_sb[:, i * 8:(i + 1) * 8, :],
                          in_=val_src[:, i * 8:(i + 1) * 8, :])
    nc.gpsimd.dma_start(out=wk_b, in_=wkb.rearrange("(t p) -> p t", p=P))
    nc.gpsimd.dma_start(out=wq_b, in_=wqb.rearrange("(t p) -> p t", p=P))
    nc.gpsimd.dma_start(out=bo_row, in_=bo.rearrange("(o c) -> o c", o=1))
    nc.gpsimd.partition_broadcast(out_ap=bo_bc, in_ap=bo_row)
    for i in range(2):
        nc.gpsimd.dma_start(out=gt_sb[:, i * 8:(i + 1) * 8, :],
                            in_=gt_src[:, i * 8:(i + 1) * 8, :])

    # PE p-state warmup: the tensor engine ramps to full clock only after
    # 3us of continuous execution.  While the first input DMAs are in
    # flight, run a chain of dummy matmuls on the memset tile so the ramp
    # completes before the first real projection, which then runs at full
    # speed instead of the 2x mid-p-state clock.
    warmP = pP.tile([P, QC], F32, tag="PP", bufs=2, name="warm")
    for i in range(88):
        nc.tensor.matmul(warmP[0:64, 0:64], warm_sb[:, 0:64],
                         warm_sb[:, 0:64], start=True, stop=True)

    # ---------------- k/q projections (copies on DVE, keeping ACT exp-only) -
    def kproj(ct, sc):
        pp = pP.tile([P, QC], F32, tag="PP", bufs=2, name=f"ppk{ct}_{sc}")
        for t in range(2):
            nc.tensor.matmul(pp, wkT_sb[:, t, ct * P:(ct + 1) * P],
                             keyT_sb[:, t, sc * QC:(sc + 1) * QC],
                             start=(t == 0), stop=(t == 1))
        nc.vector.tensor_scalar_add(out=kT[:, ct, sc * QC:(sc + 1) * QC],
                                    in0=pp, scalar1=wk_b[:, ct:ct + 1])

    def qproj(ct, sc, act=False):
        pq = pP.tile([P, QC], F32, tag="OU", bufs=2, name=f"ppq{ct}_{sc}")
        for t in range(2):
            nc.tensor.matmul(pq, wqT_sb[:, t, ct * P:(ct + 1) * P],
                             qryT_sb[:, t, sc * QC:(sc + 1) * QC],
                             start=(t == 0), stop=(t == 1))
        if act:
            # ACT copy runs in parallel with the k-proj copy on DVE, so the
            # first scores are gated by max() of the two, not their sum
            nc.scalar.activation(out=qT[:, ct, sc * QC:(sc + 1) * QC],
                                 in_=pq, func=AF.Identity,
                                 bias=wq_b[:, ct:ct + 1])
        else:
            nc.vector.tensor_scalar_add(out=qT[:, ct, sc * QC:(sc + 1) * QC],
                                        in0=pq, scalar1=wq_b[:, ct:ct + 1])

    # minimal projections for unit 0 (head 0, q-chunk 0); the rest are
    # deferred into unit 0's task list (kT/qT ct covers heads 4ct..4ct+3)
    kproj(0, 0)
    qproj(0, 0)

    # ---------------- main pipeline -----------------------------------------
    out_r = out.rearrange("(n p) d -> p n d", p=P)
    Pp = {}  # unit v -> [P psum ct0, ct1]
    Dn = {}  # unit v -> denominator psum

    def emit_scores(u, g):
        h, qc = divmod(u, NQC)
        base, hg = 32 * (h % 4), h // 4
        E = E_ring[u % 2]
        ps = pP.tile([P, 2, QC], F32, tag="SC", bufs=2, name=f"sc{u}_{g}")
        for i in range(2):
            st = 2 * g + i
            nc.tensor.matmul(
                ps[:, i, :],
                kT[base:base + 32, hg, st * P:(st + 1) * P],
                qT[base:base + 32, hg, qc * QC:(qc + 1) * QC],
                start=True, stop=True,
                tile_position=(base, 0))
        nc.scalar.activation(out=E[:, 2 * g:2 * g + 2, :], in_=ps, func=AF.Exp)

    def pchain(v, g):
        """P(v) accumulation over s-tile pair g (E(v) groups g ready)."""
        E = E_ring[v % 2]
        if g == 0:
            Pp[v] = [pP.tile([P, QC], F32, tag="PP", bufs=2,
                             name=f"pp{v}_{ct}") for ct in range(2)]
        for i in range(2):
            st = 2 * g + i
            for ct in range(2):
                nc.tensor.matmul(
                    Pp[v][ct], val_sb[:, st, ct * P:(ct + 1) * P],
                    E[:, st, :],
                    start=(st == 0), stop=(st == ST - 1),
                    skip_group_check=True)

    def dchain(v, j):
        E = E_ring[v % 2]
        if j == 0:
            Dn[v] = pP.tile([P, 512], F32, tag="OU", bufs=2, name=f"dn{v}")
        for st in range(ST):
            nc.tensor.matmul(
                Dn[v][:, j:j + 1], E[:, st, j * P:(j + 1) * P], ones_bf,
                start=(st == 0), stop=(st == ST - 1),
                skip_group_check=True)

    def cps(v):
        """P(v) psum -> SBUF bf16 (ct0 on DVE, ct1 on ACT in parallel),
        1/denominator on DVE."""
        h, qc = divmod(v, NQC)
        nc.vector.tensor_copy(
            out=P_sb[h % 2][:, 0, qc * QC:(qc + 1) * QC], in_=Pp[v][0])
        nc.scalar.activation(
            out=P_sb[h % 2][:, 1, qc * QC:(qc + 1) * QC], in_=Pp[v][1],
            func=AF.Copy)
        nc.vector.reciprocal(out=recip_t[:, h, qc * 4:(qc + 1) * 4],
                             in_=Dn[v][:, 0:4])

    def out2_pair(h, qp):
        """out psum [q, e] for query tiles 2qp..2qp+1 of head h, scaled by
        1/denom into acc; per-qt output DMA after the last head's scale."""
        po = pP.tile([P, 2, DM], F32, tag="OU", bufs=2, name=f"o2_{h}_{qp}")
        for i in range(2):
            qt = 2 * qp + i
            for ct in range(2):
                nc.tensor.matmul(
                    po[:, i, :],
                    P_sb[h % 2][:, ct, qt * P:(qt + 1) * P],
                    gt_sb[:, 2 * h + ct, :],
                    start=(ct == 0), stop=(ct == 1),
                    skip_group_check=True)
        for i in range(2):
            qt = 2 * qp + i
            nc.vector.scalar_tensor_tensor(
                out=acc[:, qt, :], in0=po[:, i, :],
                scalar=recip_t[:, h, qt:qt + 1],
                in1=(bo_bc if h == 0 else acc[:, qt, :]),
                op0=mult, op1=add)
            if h == H - 1:
                eng = nc.sync if qt % 2 == 0 else nc.scalar
                eng.dma_start(out=out_r[:, qt:qt + 1, :],
                              in_=acc[:, qt:qt + 1, :])

    def tail_unit():
        """Last unit drain: finish the ct0 chain first so its SBUF copy
        overlaps the ct1 chain (ct1 copy on ACT in parallel), then the final
        out2 pairs with per-pair DMAs."""
        v, h = NU - 1, H - 1
        E = E_ring[v % 2]

        def pl_ct(ct):
            for st in range(8, ST):
                nc.tensor.matmul(
                    Pp[v][ct], val_sb[:, st, ct * P:(ct + 1) * P],
                    E[:, st, :], start=False, stop=(st == ST - 1),
                    skip_group_check=True)

        pl_ct(0)
        nc.vector.tensor_copy(out=P_sb[h % 2][:, 0, QC:2 * QC], in_=Pp[v][0])
        for j in range(4):
            dchain(v, j)
        nc.vector.reciprocal(out=recip_t[:, h, 4:8], in_=Dn[v][:, 0:4])
        # hoist the first query-tile's ct0 partial of each pair ahead of the
        # ct1 chain (safe: exactly ONE open accumulation group per psum bank)
        pos = {}
        for qp in (2, 3):
            po = pP.tile([P, 2, DM], F32, tag="OU", bufs=2, name=f"o2t_{qp}")
            pos[qp] = po
            qt = 2 * qp
            nc.tensor.matmul(
                po[:, 0, :], P_sb[h % 2][:, 0, qt * P:(qt + 1) * P],
                gt_sb[:, 2 * h, :], start=True, stop=False,
                skip_group_check=True)
        pl_ct(1)
        for qp in (2, 3):
            o = (qp - 2) * 2 * P
            nc.scalar.activation(
                out=P_sb[h % 2][:, 1, QC + o:QC + o + 2 * P],
                in_=Pp[v][1][:, o:o + 2 * P], func=AF.Copy)
            po = pos[qp]
            qt0 = 2 * qp
            nc.tensor.matmul(
                po[:, 0, :], P_sb[h % 2][:, 1, qt0 * P:(qt0 + 1) * P],
                gt_sb[:, 2 * h + 1, :], start=False, stop=True,
                skip_group_check=True)
            for ct in range(2):
                nc.tensor.matmul(
                    po[:, 1, :],
                    P_sb[h % 2][:, ct, (qt0 + 1) * P:(qt0 + 2) * P],
                    gt_sb[:, 2 * h + ct, :],
                    start=(ct == 0), stop=(ct == 1), skip_group_check=True)
            for i in range(2):
                qt = qt0 + i
                nc.vector.scalar_tensor_tensor(
                    out=acc[:, qt, :], in0=po[:, i, :],
                    scalar=recip_t[:, h, qt:qt + 1],
                    in1=acc[:, qt, :], op0=mult, op1=add)
                if qp == 3:
                    eng = nc.scalar if qt == 6 else nc.sync
                    eng.dma_start(out=out_r[:, qt:qt + 1, :],
                                  in_=acc[:, qt:qt + 1, :])
            if qp == 2:
                nc.sync.dma_start(out=out_r[:, 4:6, :], in_=acc[:, 4:6, :])

    def build_tasks(u):
        """Half-unit-lag pipeline: unit u drains the LATE half of P(u-1),
        its denominators, copies, recip and the out2 half for the query
        chunk just copied, then starts the EARLY half of P(u)."""
        T = []
        if u == NU:
            T.append(tail_unit)
        elif 1 <= u < NU:
            v = u - 1
            h, qc = divmod(v, NQC)
            for g in range(4, 8):
                T.append(lambda g=g, v=v: pchain(v, g))
            for j in range(4):
                T.append(lambda j=j, v=v: dchain(v, j))
            T.append(lambda v=v: cps(v))
            for qp in ((0, 1) if qc == 0 else (2, 3)):
                T.append(lambda qp=qp, h=h: out2_pair(h, qp))
        if u == 0:
            # tile_wait_until marks the deferred projections as logically
            # later so the list scheduler doesn't hoist them ahead of the
            # unit-0 scores in the in-order PE queue (their key/qry chunks
            # arrive staggered; scores g0/g1 data is ready much earlier)
            def defproj(fn):
                def go():
                    with tc.tile_wait_until(0.006):
                        fn()
                return go
            for ct, sc in ((0, 1), (0, 2), (0, 3), (1, 0), (1, 1), (1, 2),
                           (1, 3)):
                T.append(defproj(lambda ct=ct, sc=sc: kproj(ct, sc)))
            for ct, sc in ((0, 1), (1, 0), (1, 1)):
                T.append(defproj(lambda ct=ct, sc=sc: qproj(ct, sc)))
        if u < NU:
            for g in range(4):
                T.append(lambda g=g, u=u: pchain(u, g))
        return T

    for u in range(NU + 1):
        T = build_tasks(u)
        if u < NU:
            ti = 0
            for g in range(8):
                emit_scores(u, g)
                upto = (len(T) * (g + 1)) // 8
                for t in T[ti:upto]:
                    t()
                ti = upto
            for t in T[ti:]:
                t()
        else:
            for t in T:
                t()


_NC_CACHE = None


def _get_nc():
    global _NC_CACHE
    if _NC_CACHE is None:
        _NC_CACHE = _build_bass()
    return _NC_CACHE


def _make_in_maps(inputs):
    f = lambda x: np.asarray(x, dtype=np.float32)
    WK, WKb = f(inputs["WK_w"]), f(inputs["WK_b"])
    WQ, WQb = f(inputs["WQ_w"]), f(inputs["WQ_b"])
    WV, WVb = f(inputs["WV_w"]), f(inputs["WV_b"])
    WO, WOb = f(inputs["WO_w"]), f(inputs["WO_b"])

    # host-folded weights: G_h = WO_h @ WV_h, shipped as gt[h*256+c, e] = G_h^T
    WO_r = WO.reshape(DM, DM, H)                  # [e, dm, h]
    gt = np.empty((H, DM, DM), np.float32)        # [h, c, e]
    for h in range(H):
        gt[h] = (WO_r[:, :, h] @ WV[h * DM:(h + 1) * DM]).T
    gt = np.ascontiguousarray(gt.reshape(H * DM, DM).astype(BF))
    # host-folded bias: bo = wob + sum_h WO_h @ bv_h  (softmax rows sum to 1)
    bvec = WVb.reshape(H, DM).T.reshape(-1)       # [dm*8 + h]
    bo = np.ascontiguousarray((WOb + WO @ bvec).astype(np.float32))

    shared = {
        "wkq": np.ascontiguousarray(
            np.concatenate([WK.T, WQ.T], axis=0).astype(BF)),
        "wkb": np.ascontiguousarray(WKb),
        "wqb": np.ascontiguousarray(WQb),
        "gt": gt,
        "bo": bo,
    }
    key_in = f(inputs["key_input"])
    qry_in = f(inputs["query_input"])
    val_in = f(inputs["value_input"])
    in_maps = []
    for c in range(N_CORES):
        b, qs = c // 2, c % 2
        in_maps.append(dict(
            shared,
            keyT=np.ascontiguousarray(key_in[b].T.astype(BF)),
            qryT=np.ascontiguousarray(
                qry_in[b, qs * QSH:(qs + 1) * QSH].T.astype(BF)),
            val_x=np.ascontiguousarray(val_in[b].astype(BF)),
        ))
    return in_maps


def _assemble(results):
    out = np.empty((B, T2, DM), dtype=np.float32)
    for c in range(N_CORES):
        b, qs = c // 2, c % 2
        out[b, qs * QSH:(qs + 1) * QSH] = results[c]["out_y"]
    return out


def run_spmd(inputs, **kwargs):
    """Run the kernel on all 8 cores; kwargs forwarded (e.g. trace=True)."""
    nc = _get_nc()
    res = run_bass_kernel_spmd(nc, _make_in_maps(inputs),
                               core_ids=list(range(N_CORES)), **kwargs)
    return res


def kernel(**inputs):
    res = run_spmd(inputs)
    return _assemble(res.results)


# revision 55
# speedup vs baseline: 1.0014x; 1.0014x over previous
"""Multi-head attention Bass/Tile kernel for 8 TRN2 NeuronCores.

Problem: nn_MultiHeadAttention (B=4, T1=T2=2048, d_model=256, d_key=32, H=8,
per-head value dim = d_model).  Reference math (no score scaling, no mask):

    k = key   @ WK^T + bk           [B, T1, 256]   (head h -> cols 32h..32h+32)
    q = query @ WQ^T + bq           [B, T2, 256]
    v = value @ WV^T + bv           [B, T1, 2048]  (head h -> cols 256h..256h+256)
    scores_h = k_h q_h^T            [T1, T2]
    attn = softmax over T1 (keys)
    emb_h = attn^T v_h              [T2, 256]
    out = emb' @ WO^T + bo          emb' channel c = d*8 + h (d outer, h inner)

Key algebraic restructure vs the direct form: since softmax weights sum to 1,

    out[q, :] = sum_h (attn_h^T @ val) @ G_h^T + bo
    G_h = WO_h @ WV_h   (host-folded weight product, [256, 256] per head)
    bo  = wob + sum_h WO_h @ bv_h   (host-folded bias)

so the per-head value projection (val @ WV_h^T) and the WO matmul collapse
into a single small per-head GEMM against the unnormalized attention-weighted
value sum P_h[c, q] = sum_s val[s, c] E_h[s, q], normalized at the end by the
per-query 1/denominator.  This removes the v-projection (64 MB intermediate,
~27 us of PE time per core) and all wo transposes from the device program.

Sharding: core c handles (batch b = c//2, query half qs = c%2) -> each core
computes the full output slice out[b, qs*1024:(qs+1)*1024, :].  No collectives.

Layout strategy: the host ships bf16 inputs pre-transposed so the device does
ZERO transposes and ZERO dtype casts:
  - keyT/qryT [256, s] channel-major (for the k/q projections)
  - val [s, 256] natural (stationary operand of the P matmul contracts over s)
  - gt [h*256+c, e] = G_h^T rows (rhs of the output GEMM)

Per-core pipeline (units u = (head, 512-query-chunk), 16 units, half-unit
software lag):
  - scores_h = kT_h^T qT_h on PE into PSUM [s, q] tiles (bf16, tile_position
    packs the 32-row stationary into PE quadrant rows 32*(h%4)).
  - E = exp(scores) on ACT (no max subtraction; fp32 range is plenty), the
    only ACT work in the kernel so the Exp table is loaded exactly once.
  - P(u): 32 chained matmuls val-stationary x E -> psum [c, 512q]; the early
    half runs inside unit u itself, the late half + denominators (ones-
    matmul, out free size 1) in unit u+1, so the pipeline drains half a unit
    after the last score.
  - copies P->SBUF bf16 (ct0 on DVE, ct1 on ACT) + reciprocal on DVE in
    unit u+1, then out2(h): psum [q, e] = P_h^T G_h^T, scaled by recip and
    accumulated into acc on DVE (scalar_tensor_tensor), bias at h==0.
  - a chain of dummy matmuls on a memset tile pre-ramps the PE p-state
    during the initial DMAs, so real work starts at the full 2.4 GHz clock
    instead of spending its first 3 us at the mid-p-state clock.
  - k/q projections for the second head group are deferred into unit 0's
    task list, marked logically-later via tile_wait_until so the Tile list
    scheduler cannot hoist them ahead of the unit-0 scores in the in-order
    PE queue (their staggered key chunks would stall the pipe); DMA chunks
    are interleaved across the sync/scalar HWDGE queues in compute need-
    order, with gt/biases on the gpsimd SWDGE queue.
  - tail: the last unit finishes its ct0 chain first (copy overlaps ct1),
    ct1 copy rides ACT, and the final output DMAs are split/queued so the
    last transfer is a single 128-query tile.

Measured (TimelineSim cost model, the graded metric): 196038 ns vs 304140 ns
for the previous kernel (1.55x), rel err 8.0e-3 (budget 2e-2).  PE busy is
~184 us = the algorithmic floor for this formulation (scores 131k cycles +
P 262k + out2 33k + proj 12k at full MAC efficiency); residual time is the
irreducible first-DMA latency chain (~5 us) and the final
scale->output-DMA->drain chain (~4.5 us).

kernel(**inputs) takes the FULL unsharded inputs and returns the full output.
"""

import numpy as np
import ml_dtypes
from contextlib import ExitStack

import concourse.bass as bass
import concourse.bacc as bacc
import concourse.mybir as mybir
import concourse.tile as tile
from concourse.bass_utils import run_bass_kernel_spmd

P = 128
B, T1, T2, DM, DK, H = 4, 2048, 2048, 256, 32, 8
QSH = T2 // 2  # queries per core
N_CORES = 8

F32 = mybir.dt.float32
BF16 = mybir.dt.bfloat16
AF = mybir.ActivationFunctionType
BF = ml_dtypes.bfloat16

ST = T1 // P        # 16 key/seq tiles
QT = QSH // P       # 8 query tiles per core
QC = 512            # query chunk (PSUM free dim)
NQC = QSH // QC     # 2 query chunks
NU = H * NQC        # 16 pipeline units


def _build_bass():
    nc = bacc.Bacc("TRN2", target_bir_lowering=False, debug=False)

    keyT = nc.dram_tensor("keyT", [DM, T1], BF16, kind="ExternalInput").ap()
    qryT = nc.dram_tensor("qryT", [DM, QSH], BF16, kind="ExternalInput").ap()
    val = nc.dram_tensor("val_x", [T1, DM], BF16, kind="ExternalInput").ap()
    wkq = nc.dram_tensor("wkq", [2 * DM, DM], BF16, kind="ExternalInput").ap()
    wkb = nc.dram_tensor("wkb", [DM], F32, kind="ExternalInput").ap()
    wqb = nc.dram_tensor("wqb", [DM], F32, kind="ExternalInput").ap()
    gt = nc.dram_tensor("gt", [H * DM, DM], BF16, kind="ExternalInput").ap()
    bo = nc.dram_tensor("bo", [DM], F32, kind="ExternalInput").ap()
    out = nc.dram_tensor("out_y", [QSH, DM], F32, kind="ExternalOutput").ap()

    with tile.TileContext(nc, pool_alloc_mode="queue") as tc:
        with ExitStack() as ctx:
            _body(ctx, tc, keyT, qryT, val, wkq, wkb, wqb, gt, bo, out)
    nc.compile()
    return nc


def _body(ctx, tc, keyT, qryT, val, wkq, wkb, wqb, gt, bo, out):
    nc = tc.nc
    mult, add = mybir.AluOpType.mult, mybir.AluOpType.add
    consts = ctx.enter_context(tc.tile_pool(name="consts", bufs=1))
    main = ctx.enter_context(tc.tile_pool(name="main", bufs=1))
    # One PSUM pool, 8 banks total:
    #   SC: [128,2,512] f32 x2  (4 banks) score tiles
    #   PP: [128,512]  f32 x2   (2 banks) k-proj, then P accumulators
    #   OU: [128,512]  f32 x2   (2 banks) q-proj, denominators, out2 psums
    pP = ctx.enter_context(tc.tile_pool(name="pP", bufs=1, space="PSUM"))

    # touch the ACT Exp table early; exp is the ONLY ACT op in the kernel so
    # the table is loaded exactly once, off the critical path
    actwarm = consts.tile([1, 1], F32)
    nc.scalar.activation(out=actwarm, in_=actwarm, func=AF.Exp)
    warm_sb = consts.tile([P, 64], BF16)
    nc.vector.memset(warm_sb, 0.0)
    ones_bf = consts.tile([P, 1], BF16)
    nc.vector.memset(ones_bf, 1.0)
    wk_b = consts.tile([P, 2], F32)
    wq_b = consts.tile([P, 2], F32)
    bo_row = consts.tile([1, DM], F32)
    bo_bc = consts.tile([P, DM], F32)

    # persistent SBUF tensors
    keyT_sb = main.tile([P, 2, T1], BF16)        # [c%128, c//128, s]
    qryT_sb = main.tile([P, 2, QSH], BF16)
    wkq_sb = main.tile([P, 4, DM], BF16)         # [c%128, c//128 (+2 for wq), ch]
    wkT_sb = wkq_sb[:, 0:2, :]
    wqT_sb = wkq_sb[:, 2:4, :]
    val_sb = main.tile([P, ST, DM], BF16)        # [s%128, s//128, c]
    gt_sb = main.tile([P, 2 * H, DM], BF16)      # [c%128, 2h+(c//128), e]
    kT = main.tile([P, 2, T1], BF16)             # [32*(h%4)+d, h//4, s]
    qT = main.tile([P, 2, QSH], BF16)
    E_ring = [main.tile([P, ST, QC], BF16, name=f"Er{i}") for i in range(2)]
    P_sb = [main.tile([P, 2, QSH], BF16, name=f"Psb{i}") for i in range(2)]
    recip_t = main.tile([P, H, QT], F32)         # [q%128, h, q//128] = 1/denom
    acc = main.tile([P, QT, DM], F32)            # output accumulator [q, e]

    # ---------------- DMA loads, priority-ordered ---------------------------
    keyT_src = keyT.rearrange("(t p) s -> p t s", p=P)
    qryT_src = qryT.rearrange("(t p) s -> p t s", p=P)
    val_src = val.rearrange("(n p) c -> p n c", p=P)
    gt_src = gt.rearrange("(n p) e -> p n e", p=P)
    # the HWDGE queues drain in near-alternation on the shared HWDGE /
    # DMA_ENGINES devices; assign chunks so that alternation matches the
    # order compute needs them: wkq, key0, qry0, qry1, key1, key3, key2, ...
    # (key0 gates the very first matmul, so it rides the scalar queue whose
    # first slot clears right after wkq).  gt goes via the gpsimd SWDGE
    # queue, off the HWDGE device entirely.
    nc.sync.dma_start(out=wkq_sb, in_=wkq.rearrange("(t p) c -> p t c", p=P))
    nc.scalar.dma_start(out=keyT_sb[:, :, 0:QC], in_=keyT_src[:, :, 0:QC])
    nc.sync.dma_start(out=qryT_sb[:, :, 0:QC], in_=qryT_src[:, :, 0:QC])
    nc.scalar.dma_start(out=qryT_sb[:, :, QC:2 * QC],
                        in_=qryT_src[:, :, QC:2 * QC])
    nc.sync.dma_start(out=keyT_sb[:, :, QC:2 * QC],
                      in_=keyT_src[:, :, QC:2 * QC])
    nc.scalar.dma_start(out=keyT_sb[:, :, 3 * QC:4 * QC],
                        in_=keyT_src[:, :, 3 * QC:4 * QC])
    nc.sync.dma_start(out=keyT_sb[:, :, 2 * QC:3 * QC],
                      in_=keyT_src[:, :, 2 * QC:3 * QC])
    for i in range(2):
        nc.sync.dma_start(out=val_sb[:, i * 8:(i + 1) * 8, :],
                          in_=val_src[:, i * 8:(i + 1) * 8, :])
    nc.gpsimd.dma_start(out=wk_b, in_=wkb.rearrange("(t p) -> p t", p=P))
    nc.gpsimd.dma_start(out=wq_b, in_=wqb.rearrange("(t p) -> p t", p=P))
    nc.gpsimd.dma_start(out=bo_row, in_=bo.rearrange("(o c) -> o c", o=1))
    nc.gpsimd.partition_broadcast(out_ap=bo_bc, in_ap=bo_row)
    for i in range(2):
        nc.gpsimd.dma_start(out=gt_sb[:, i * 8:(i + 1) * 8, :],
                            in_=gt_src[:, i * 8:(i + 1) * 8, :])

    # PE p-state warmup: the tensor engine ramps to full clock only after
    # 3us of continuous execution.  While the first input DMAs are in
    # flight, run a chain of dummy matmuls on the memset tile so the ramp
    # completes before the first real projection, which then runs at full
    # speed instead of the 2x mid-p-state clock.
    warmP = pP.tile([P, QC], F32, tag="PP", bufs=2, name="warm")
    for i in range(88):
        nc.tensor.matmul(warmP[0:64, 0:64], warm_sb[:, 0:64],
                         warm_sb[:, 0:64], start=True, stop=True)

    # ---------------- k/q projections (copies on DVE, keeping ACT exp-only) -
    def kproj(ct, sc):
        pp = pP.tile([P, QC], F32, tag="PP", bufs=2, name=f"ppk{ct}_{sc}")
        for t in range(2):
            nc.tensor.matmul(pp, wkT_sb[:, t, ct * P:(ct + 1) * P],
                             keyT_sb[:, t, sc * QC:(sc + 1) * QC],
                             start=(t == 0), stop=(t == 1))
        nc.vector.tensor_scalar_add(out=kT[:, ct, sc * QC:(sc + 1) * QC],
                                    in0=pp, scalar1=wk_b[:, ct:ct + 1])

    def qproj(ct, sc, act=False):
        pq = pP.tile([P, QC], F32, tag="OU", bufs=2, name=f"ppq{ct}_{sc}")
        for t in range(2):
            nc.tensor.matmul(pq, wqT_sb[:, t, ct * P:(ct + 1) * P],
                             qryT_sb[:, t, sc * QC:(sc + 1) * QC],
                             start=(t == 0), stop=(t == 1))
        if act:
            # ACT copy runs in parallel with the k-proj copy on DVE, so the
            # first scores are gated by max() of the two, not their sum
            nc.scalar.activation(out=qT[:, ct, sc * QC:(sc + 1) * QC],
                                 in_=pq, func=AF.Identity,
                                 bias=wq_b[:, ct:ct + 1])
        else:
            nc.vector.tensor_scalar_add(out=qT[:, ct, sc * QC:(sc + 1) * QC],
                                        in0=pq, scalar1=wq_b[:, ct:ct + 1])

    # minimal projections for unit 0 (head 0, q-chunk 0); the rest are
    # deferred into unit 0's task list (kT/qT ct covers heads 4ct..4ct+3)
    kproj(0, 0)
    qproj(0, 0)

    # ---------------- main pipeline -----------------------------------------
    out_r = out.rearrange("(n p) d -> p n d", p=P)
    Pp = {}  # unit v -> [P psum ct0, ct1]
    Dn = {}  # unit v -> denominator psum

    def emit_scores(u, g):
        h, qc = divmod(u, NQC)
        base, hg = 32 * (h % 4), h // 4
        E = E_ring[u % 2]
        ps = pP.tile([P, 2, QC], F32, tag="SC", bufs=2, name=f"sc{u}_{g}")
        for i in range(2):
            st = 2 * g + i
            nc.tensor.matmul(
                ps[:, i, :],
                kT[base:base + 32, hg, st * P:(st + 1) * P],
                qT[base:base + 32, hg, qc * QC:(qc + 1) * QC],
                start=True, stop=True,
                tile_position=(base, 0))
        nc.scalar.activation(out=E[:, 2 * g:2 * g + 2, :], in_=ps, func=AF.Exp)

    def pchain(v, g):
        """P(v) accumulation over s-tile pair g (E(v) groups g ready)."""
        E = E_ring[v % 2]
        if g == 0:
            Pp[v] = [pP.tile([P, QC], F32, tag="PP", bufs=2,
                             name=f"pp{v}_{ct}") for ct in range(2)]
        for i in range(2):
            st = 2 * g + i
            for ct in range(2):
                nc.tensor.matmul(
                    Pp[v][ct], val_sb[:, st, ct * P:(ct + 1) * P],
                    E[:, st, :],
                    start=(st == 0), stop=(st == ST - 1),
                    skip_group_check=True)

    def dchain(v, j):
        E = E_ring[v % 2]
        if j == 0:
            Dn[v] = pP.tile([P, 512], F32, tag="OU", bufs=2, name=f"dn{v}")
        for st in range(ST):
            nc.tensor.matmul(
                Dn[v][:, j:j + 1], E[:, st, j * P:(j + 1) * P], ones_bf,
                start=(st == 0), stop=(st == ST - 1),
                skip_group_check=True)

    def cps(v):
        """P(v) psum -> SBUF bf16 (ct0 on DVE, ct1 on ACT in parallel),
        1/denominator on DVE."""
        h, qc = divmod(v, NQC)
        nc.vector.tensor_copy(
            out=P_sb[h % 2][:, 0, qc * QC:(qc + 1) * QC], in_=Pp[v][0])
        nc.scalar.activation(
            out=P_sb[h % 2][:, 1, qc * QC:(qc + 1) * QC], in_=Pp[v][1],
            func=AF.Copy)
        nc.vector.reciprocal(out=recip_t[:, h, qc * 4:(qc + 1) * 4],
                             in_=Dn[v][:, 0:4])

    def out2_pair(h, qp):
        """out psum [q, e] for query tiles 2qp..2qp+1 of head h, scaled by
        1/denom into acc; per-qt output DMA after the last head's scale."""
        po = pP.tile([P, 2, DM], F32, tag="OU", bufs=2, name=f"o2_{h}_{qp}")
        for i in range(2):
            qt = 2 * qp + i
            for ct in range(2):
                nc.tensor.matmul(
                    po[:, i, :],
                    P_sb[h % 2][:, ct, qt * P:(qt + 1) * P],
                    gt_sb[:, 2 * h + ct, :],
                    start=(ct == 0), stop=(ct == 1),
                    skip_group_check=True)
        for i in range(2):
            qt = 2 * qp + i
            nc.vector.scalar_tensor_tensor(
                out=acc[:, qt, :], in0=po[:, i, :],
                scalar=recip_t[:, h, qt:qt + 1],
                in1=(bo_bc if h == 0 else acc[:, qt, :]),
                op0=mult, op1=add)
            if h == H - 1:
                eng = nc.sync if qt % 2 == 0 else nc.scalar
                eng.dma_start(out=out_r[:, qt:qt + 1, :],
                              in_=acc[:, qt:qt + 1, :])

    def tail_unit():
        """Last unit drain: finish the ct0 chain first so its SBUF copy
        overlaps the ct1 chain (ct1 copy on ACT in parallel), then the final
        out2 pairs with per-pair DMAs."""
        v, h = NU - 1, H - 1
        E = E_ring[v % 2]

        def pl_ct(ct):
            for st in range(8, ST):
                nc.tensor.matmul(
                    Pp[v][ct], val_sb[:, st, ct * P:(ct + 1) * P],
                    E[:, st, :], start=False, stop=(st == ST - 1),
                    skip_group_check=True)

        pl_ct(0)
        nc.vector.tensor_copy(out=P_sb[h % 2][:, 0, QC:2 * QC], in_=Pp[v][0])
        pl_ct(1)
        for j in range(4):
            dchain(v, j)
        nc.vector.reciprocal(out=recip_t[:, h, 4:8], in_=Dn[v][:, 0:4])
        for qp in (2, 3):
            o = (qp - 2) * 2 * P
            nc.scalar.activation(
                out=P_sb[h % 2][:, 1, QC + o:QC + o + 2 * P],
                in_=Pp[v][1][:, o:o + 2 * P], func=AF.Copy)
            po = pP.tile([P, 2, DM], F32, tag="OU", bufs=2, name=f"o2t_{qp}")
            for i in range(2):
                qt = 2 * qp + i
                for ct in range(2):
                    nc.tensor.matmul(
                        po[:, i, :], P_sb[h % 2][:, ct, qt * P:(qt + 1) * P],
                        gt_sb[:, 2 * h + ct, :],
                        start=(ct == 0), stop=(ct == 1), skip_group_check=True)
            for i in range(2):
                qt = 2 * qp + i
                nc.vector.scalar_tensor_tensor(
                    out=acc[:, qt, :], in0=po[:, i, :],
                    scalar=recip_t[:, h, qt:qt + 1],
                    in1=acc[:, qt, :], op0=mult, op1=add)
                if qp == 3:
                    eng = nc.scalar if qt == 6 else nc.sync
                    eng.dma_start(out=out_r[:, qt:qt + 1, :],
                                  in_=acc[:, qt:qt + 1, :])
            if qp == 2:
                nc.sync.dma_start(out=out_r[:, 4:6, :], in_=acc[:, 4:6, :])

    def build_tasks(u):
        """Half-unit-lag pipeline: unit u drains the LATE half of P(u-1),
        its denominators, copies, recip and the out2 half for the query
        chunk just copied, then starts the EARLY half of P(u)."""
        T = []
        if u == NU:
            T.append(tail_unit)
        elif 1 <= u < NU:
            v = u - 1
            h, qc = divmod(v, NQC)
            for g in range(4, 8):
                T.append(lambda g=g, v=v: pchain(v, g))
            for j in range(4):
                T.append(lambda j=j, v=v: dchain(v, j))
            T.append(lambda v=v: cps(v))
            for qp in ((0, 1) if qc == 0 else (2, 3)):
                T.append(lambda qp=qp, h=h: out2_pair(h, qp))
        if u == 0:
            # tile_wait_until marks the deferred projections as logically
            # later so the list scheduler doesn't hoist them ahead of the
            # unit-0 scores in the in-order PE queue (their key/qry chunks
            # arrive staggered; scores g0/g1 data is ready much earlier)
            def defproj(fn):
                def go():
                    with tc.tile_wait_until(0.006):
                        fn()
                return go
            for ct, sc in ((0, 1), (0, 2), (0, 3), (1, 0), (1, 1), (1, 2),
                           (1, 3)):
                T.append(defproj(lambda ct=ct, sc=sc: kproj(ct, sc)))
            for ct, sc in ((0, 1), (1, 0), (1, 1)):
                T.append(defproj(lambda ct=ct, sc=sc: qproj(ct, sc)))
        if u < NU:
            for g in range(4):
                T.append(lambda g=g, u=u: pchain(u, g))
        return T

    for u in range(NU + 1):
        T = build_tasks(u)
        if u < NU:
            ti = 0
            for g in range(8):
                emit_scores(u, g)
                upto = (len(T) * (g + 1)) // 8
                for t in T[ti:upto]:
                    t()
                ti = upto
            for t in T[ti:]:
                t()
        else:
            for t in T:
                t()


_NC_CACHE = None


def _get_nc():
    global _NC_CACHE
    if _NC_CACHE is None:
        _NC_CACHE = _build_bass()
    return _NC_CACHE


def _make_in_maps(inputs):
    f = lambda x: np.asarray(x, dtype=np.float32)
    WK, WKb = f(inputs["WK_w"]), f(inputs["WK_b"])
    WQ, WQb = f(inputs["WQ_w"]), f(inputs["WQ_b"])
    WV, WVb = f(inputs["WV_w"]), f(inputs["WV_b"])
    WO, WOb = f(inputs["WO_w"]), f(inputs["WO_b"])

    # host-folded weights: G_h = WO_h @ WV_h, shipped as gt[h*256+c, e] = G_h^T
    WO_r = WO.reshape(DM, DM, H)                  # [e, dm, h]
    gt = np.empty((H, DM, DM), np.float32)        # [h, c, e]
    for h in range(H):
        gt[h] = (WO_r[:, :, h] @ WV[h * DM:(h + 1) * DM]).T
    gt = np.ascontiguousarray(gt.reshape(H * DM, DM).astype(BF))
    # host-folded bias: bo = wob + sum_h WO_h @ bv_h  (softmax rows sum to 1)
    bvec = WVb.reshape(H, DM).T.reshape(-1)       # [dm*8 + h]
    bo = np.ascontiguousarray((WOb + WO @ bvec).astype(np.float32))

    shared = {
        "wkq": np.ascontiguousarray(
            np.concatenate([WK.T, WQ.T], axis=0).astype(BF)),
        "wkb": np.ascontiguousarray(WKb),
        "wqb": np.ascontiguousarray(WQb),
        "gt": gt,
        "bo": bo,
    }
    key_in = f(inputs["key_input"])
    qry_in = f(inputs["query_input"])
    val_in = f(inputs["value_input"])
    in_maps = []
    for c in range(N_CORES):
        b, qs = c // 2, c % 2
        in_maps.append(dict(
            shared,
            keyT=np.ascontiguousarray(key_in[b].T.astype(BF)),
            qryT=np.ascontiguousarray(
                qry_in[b, qs * QSH:(qs + 1) * QSH].T.astype(BF)),
            val_x=np.ascontiguousarray(val_in[b].astype(BF)),
        ))
    return in_maps


def _assemble(results):
    out = np.empty((B, T2, DM), dtype=np.float32)
    for c in range(N_CORES):
        b, qs = c // 2, c % 2
        out[b, qs * QSH:(qs + 1) * QSH] = results[c]["out_y"]
    return out


def run_spmd(inputs, **kwargs):
    """Run the kernel on all 8 cores; kwargs forwarded (e.g. trace=True)."""
    nc = _get_nc()
    res = run_bass_kernel_spmd(nc, _make_in_maps(inputs),
                               core_ids=list(range(N_CORES)), **kwargs)
    return res


def kernel(**inputs):
    res = run_spmd(inputs)
    return _assemble(res.results)
